# revision 2
# baseline (speedup 1.0000x reference)
"""Trainium2 Bass kernel v2 for nn_BoundaryLoss (8-core SPMD).

Design:
- Host marshals per-core inputs (downsampled preds in f16 channel-inner layout,
  f16 embedding table as gather source, per-core pos-memory slice).
- Device: masks -> local counts -> local first-k selection (crossing search in
  flipped orientation; all values through PE are exact per probe) -> indirect
  gathers with per-slot column offsets -> normalize -> f16 payload -> ONE
  AllGather of [767,128] f16 (counts ride along as metadata row) -> compose
  global first-k tables via a second indirect gather into statically-prefilled
  merged tiles -> f16 transposes + f16 sims matmuls -> pairwise relu-sum
  split across DVE (4x mode) / Scalar / GpSimd -> partial loss out.
- Host sums the 8 partial losses.

PSUM plan (8 banks of 2KB):
  misc   [128,128] f32  small matmul outs, manual col slices      1 bank
  chain{0,1,2} [128,512] f32  crossing-search per-chunk (j%3)     3 banks
  tsp{0,1} [128,128] f16  PE transposes                           2 banks
  negsim [100,1000] f32                                           2 banks
  possim [100,125] f32  shares tag chain0 (post-collective)       0 banks
"""

import json
import sys
import types

import numpy as np

# ---------------------------------------------------------------------------
# Workarounds (walrus single-wait, semaphore range clear, axon ntff hook)
# ---------------------------------------------------------------------------


def _split_multiwaits_json(bir_bytes: bytes) -> bytes:
    j = json.loads(bir_bytes)
    ctr = 0
    changed = False
    for f in j.get("functions", []):
        for bb in f.get("blocks", []):
            new_insts = []
            for inst in bb.get("instructions", []):
                si = inst.get("sync_info")
                ow = (si or {}).get("on_wait") or []
                if len(ow) > 1:
                    changed = True
                    for w in ow[:-1]:
                        ctr += 1
                        new_insts.append(
                            {
                                "debug": inst.get("debug", 0),
                                "engine": inst["engine"],
                                "ins": [],
                                "outs": [],
                                "name": f"I-wsplit-{ctr}",
                                "opcode": "EventSemaphore",
                                "sync_info": {"on_update": [], "on_wait": [w]},
                            }
                        )
                    si["on_wait"] = [ow[-1]]
                new_insts.append(inst)
            bb["instructions"] = new_insts
    if not changed:
        return bir_bytes
    return json.dumps(j).encode()


_patched = False


def _install_patches():
    global _patched
    if _patched:
        return
    from concourse import bass as _bass

    _orig = _bass.Bass.to_json_bytes

    def _to_json_bytes(self):
        return _split_multiwaits_json(_orig(self))

    _bass.Bass.to_json_bytes = _to_json_bytes

    from concourse.bass import SemaphoreHandle as _SH, compact_to_ranges as _ctr

    def _clear_and_free(self, sems):
        if not sems:
            return
        sem_nums = [s.num if isinstance(s, _SH) else s for s in sems]
        for sem_range in _ctr(sem_nums):
            assert self._state.free_isdisjoint(sem_range)
            lo = sem_range.start
            while lo < sem_range.stop:
                hi = min(lo + 3, sem_range.stop)
                sub = range(lo, hi)
                self.gpsimd.dma_reset(sub)
                self.gpsimd.sem_clear(sub)
                lo = hi
        self._state.prepend_free_semaphores(sem_nums)
        for poison_set in self._tile_sem_poison_stack:
            poison_set.update(sem_nums)

    _bass.Bass.clear_and_free_semaphores = _clear_and_free

    try:
        import antenv

        if "antenv.axon_hooks" not in sys.modules:
            m = types.ModuleType("antenv.axon_hooks")
            _store = {}
            m.set_axon_ntff_profile_hook = lambda h: _store.__setitem__("h", h)
            m.get_axon_ntff_profile_hook = lambda: _store.get("h")
            sys.modules["antenv.axon_hooks"] = m
            antenv.axon_hooks = m
            try:
                from trn_agent_boot.trn_boot import _ntff_profile_via_ctypes

                m.set_axon_ntff_profile_hook(
                    _ntff_profile_via_ctypes("/opt/axon/libaxon_pjrt.so")
                )
            except Exception:
                pass
    except Exception:
        pass
    _patched = True


# ---------------------------------------------------------------------------
# Problem constants
# ---------------------------------------------------------------------------
B, C, H, W = 8, 21, 512, 512
D = 128
M = 1000
KP = M // 3  # 333
KA = M // 10  # 100
MARGIN = 0.2
NPIX = 128 * 128
NCORES = 8
PC = M // NCORES  # 125 pos columns per core

PAY = 767  # payload rows: 0 meta | 1:101 anc | 101:434 pos | 434:767 neg
SEC = [1, 101, 434]
GA = PAY * NCORES

# pairwise column split across engines
DVE_N = 63
ACT_N = 62
POOL_N = PC - DVE_N - ACT_N
USE_POOL_PAIRWISE = False
MULTI_OFFSET_GATHER = False

TRACE = False
LAST_EXEC_NS = None

_cache = {}


def _build_module():
    from concourse import bass, tile
    import concourse.mybir as mybir

    dt = mybir.dt
    F32 = dt.float32
    F16 = dt.float16
    I32 = dt.int32
    Alu = mybir.AluOpType
    Act = mybir.ActivationFunctionType
    AX = mybir.AxisListType.X

    nc = bass.Bass(
        trn_type="TRN2", target_bir_lowering=False, debug=False, num_devices=NCORES
    )

    # ---- I/O ----
    preds_in = nc.dram_tensor("preds_t", [128, C * 128], F16, kind="ExternalInput").ap()
    gts_in = nc.dram_tensor("gts_t", [128, 128], I32, kind="ExternalInput").ap()
    embp = nc.dram_tensor("embp", [NPIX, D], F16, kind="ExternalInput").ap()
    posown_in = nc.dram_tensor("posown", [PC, D], F32, kind="ExternalInput").ap()
    negmem_in = nc.dram_tensor("negmem", [M, D], F32, kind="ExternalInput").ap()
    identf_in = nc.dram_tensor("identf", [128, 128], F32, kind="ExternalInput").ap()
    trils_in = nc.dram_tensor("trils", [128, 128], F32, kind="ExternalInput").ap()
    riota1_in = nc.dram_tensor("riota1", [128, 1], F32, kind="ExternalInput").ap()
    rowiota_in = nc.dram_tensor("rowiota", [128, 1], F32, kind="ExternalInput").ap()
    iotaf_in = nc.dram_tensor("iotaf", [128, 128], F32, kind="ExternalInput").ap()
    siota3_in = nc.dram_tensor("siota3", [128, 3], F32, kind="ExternalInput").ap()
    shift8_in = nc.dram_tensor("shift8", [8, 8], F32, kind="ExternalInput").ap()
    sio5x8_in = nc.dram_tensor("sio5x8", [128, 40], F32, kind="ExternalInput").ap()
    sio5raw_in = nc.dram_tensor("sio5raw", [128, 5], F32, kind="ExternalInput").ap()
    sioadj_in = nc.dram_tensor("sioadj", [128, 5], F32, kind="ExternalInput").ap()
    kvec_in = nc.dram_tensor("kvec", [1, 3], F32, kind="ExternalInput").ap()
    out_d = nc.dram_tensor("out", [1, 1], F32, kind="ExternalOutput").ap()

    pay_d = nc.dram_tensor("payload", [PAY, D], F16).ap()
    gath_d = nc.dram_tensor("gath", [GA, D], F16, addr_space="Shared").ap()

    groups = [list(range(NCORES))]

    with tile.TileContext(nc) as tc:
        with tc.tile_pool(name="cst", bufs=1) as cst, \
             tc.tile_pool(name="wk", bufs=1) as wk, \
             tc.tile_pool(name="ps", bufs=1, space="PSUM") as ps:

            # ---------- constant / input loads ----------
            P = wk.tile([128, C * 128], F16, name="P")
            nc.sync.dma_start(P[:], preds_in)
            G = wk.tile([128, 128], I32, name="G")
            nc.sync.dma_start(G[:], gts_in)
            posl = wk.tile([PC, 128], F32, name="posl")
            nc.sync.dma_start(posl[:], posown_in)
            ng = []
            for i in range(8):
                r0 = 128 * i
                rn = min(128, M - r0)
                t = wk.tile([128, 128], F32, name=f"ng{i}")
                nc.sync.dma_start(t[0:rn, :], negmem_in[r0 : r0 + rn, :])
                ng.append(t)
            identf = cst.tile([128, 128], F32, name="identf")
            nc.sync.dma_start(identf[:], identf_in)
            trils = cst.tile([128, 128], F32, name="trils")
            nc.sync.dma_start(trils[:], trils_in)
            riota1 = cst.tile([128, 1], F32, name="riota1")
            nc.sync.dma_start(riota1[:], riota1_in)
            rowiota = cst.tile([128, 1], F32, name="rowiota")
            nc.sync.dma_start(rowiota[:], rowiota_in)
            iotaF = cst.tile([128, 128], F32, name="iotaF")
            nc.sync.dma_start(iotaF[:], iotaf_in)
            siota3 = cst.tile([128, 3], F32, name="siota3")
            nc.sync.dma_start(siota3[:], siota3_in)
            shift8 = cst.tile([8, 8], F32, name="shift8")
            nc.sync.dma_start(shift8[:], shift8_in)
            sio5x8 = cst.tile([128, 40], F32, name="sio5x8")
            nc.sync.dma_start(sio5x8[:], sio5x8_in)
            sio5raw = cst.tile([128, 5], F32, name="sio5raw")
            nc.sync.dma_start(sio5raw[:], sio5raw_in)
            sioadj = cst.tile([128, 5], F32, name="sioadj")
            nc.sync.dma_start(sioadj[:], sioadj_in)
            kvec = cst.tile([1, 3], F32, name="kvec")
            nc.sync.dma_start(kvec[:], kvec_in)

            zeros = cst.tile([128, 128], F32, name="zeros")
            nc.vector.memset(zeros[:], 0.0)
            onesc = cst.tile([128, 1], F32, name="onesc")
            nc.vector.memset(onesc[:], 1.0)
            onesr = cst.tile([1, 128], F32, name="onesr")
            nc.vector.memset(onesr[:], 1.0)

            # shared PSUM tiles
            misc = ps.tile([128, 160], F32, name="misc", tag="misc")
            chains = [ps.tile([128, 512], F32, name=f"chain{g}", tag=f"chain{g}")
                      for g in range(3)]
            negsim = ps.tile([100, M], F32, name="negsim", tag="negsim")
            possim = ps.tile([100, PC], F32, name="possim", tag="chain0")

            # ---------- merged-table tile (compose gather target) ----------
            # mega slices: 0=anc, 1=pos(own 125), 2..4=neg chunks 0..2
            mega = wk.tile([128, 5, 128], F16, name="mega")
            nc.vector.memset(mega[:, 0, :], 0.0)

            # ---------- masks ----------
            Pv = P[:].rearrange("p (f c) -> p f c", c=C)
            mx = wk.tile([128, 128], F16, name="mx")
            nc.vector.tensor_reduce(mx[:], Pv[:, :, 1:C], axis=AX, op=Alu.max)
            ch0 = Pv[:, :, 0:1].rearrange("p f one -> p (f one)")
            predm = wk.tile([128, 128], F32, name="predm")
            nc.vector.tensor_tensor(out=predm[:], in0=mx[:], in1=ch0, op=Alu.is_gt)
            t1m = wk.tile([128, 128], F32, name="t1m")
            nc.vector.tensor_scalar(out=t1m[:], in0=G[:], scalar1=0.0, scalar2=None,
                                    op0=Alu.not_equal)
            t2m = wk.tile([128, 128], F32, name="t2m")
            nc.vector.tensor_scalar(out=t2m[:], in0=G[:], scalar1=255.0, scalar2=None,
                                    op0=Alu.not_equal)
            e0m = wk.tile([128, 128], F32, name="e0m")
            nc.vector.tensor_scalar(out=e0m[:], in0=G[:], scalar1=0.0, scalar2=None,
                                    op0=Alu.is_equal)
            gtm = wk.tile([128, 128], F32, name="gtm")
            nc.vector.tensor_tensor(out=gtm[:], in0=t1m[:], in1=t2m[:], op=Alu.mult)
            npredm = wk.tile([128, 128], F32, name="npredm")
            nc.vector.tensor_scalar(out=npredm[:], in0=predm[:], scalar1=-1.0,
                                    scalar2=1.0, op0=Alu.mult, op1=Alu.add)
            am = wk.tile([128, 128], F32, name="am")
            nc.vector.tensor_tensor(out=am[:], in0=predm[:], in1=gtm[:], op=Alu.mult)
            pm = wk.tile([128, 128], F32, name="pm")
            nc.vector.tensor_tensor(out=pm[:], in0=gtm[:], in1=npredm[:], op=Alu.mult)
            nm = wk.tile([128, 128], F32, name="nm")
            nc.vector.tensor_tensor(out=nm[:], in0=predm[:], in1=e0m[:], op=Alu.mult)
            masks = [am, pm, nm]

            # ---------- local counts (clamped) -> payload meta ----------
            rs3 = wk.tile([128, 3], F32, name="rs3")
            for mi, mk in enumerate(masks):
                nc.vector.tensor_reduce(rs3[:, mi : mi + 1], mk[:], axis=AX, op=Alu.add)
            cnt_ps = misc[0:1, 0:3]
            nc.tensor.matmul(cnt_ps, onesc[:], rs3[:], start=True, stop=True)
            cntloc = wk.tile([1, 3], F32, name="cntloc")
            nc.vector.tensor_tensor(out=cntloc[:], in0=cnt_ps, in1=kvec[:],
                                    op=Alu.min)
            cl16 = wk.tile([1, 3], F16, name="cl16")
            nc.vector.tensor_copy(cl16[:], cntloc[:])
            nc.sync.dma_start(pay_d[0:1, 0:3], cl16[:])
            clb_ps = misc[:, 3:6]
            nc.tensor.matmul(clb_ps, onesr[:], cntloc[:], start=True, stop=True)
            clb = wk.tile([128, 3], F32, name="clb")
            nc.scalar.copy(clb[:], clb_ps)

            # ---------- local selection: scan + crossing search ----------
            rowsum3 = wk.tile([128, 3], F32, name="rowsum3")
            scns = []
            for mi, mk in enumerate(masks):
                scn = wk.tile([128, 128], F32, name=f"scn{mi}")
                nc.vector.tensor_tensor_scan(scn[:], mk[:], zeros[:], 0.0,
                                             Alu.add, Alu.add)
                nc.vector.tensor_copy(rowsum3[:, mi : mi + 1], scn[:, 127:128])
                scns.append(scn)
            ro_ps = misc[:, 6:9]
            nc.tensor.matmul(ro_ps, trils[:], rowsum3[:], start=True, stop=True)
            rowoff3 = wk.tile([128, 3], F32, name="rowoff3")
            nc.scalar.copy(rowoff3[:], ro_ps)
            Pgs = []
            for mi in range(3):
                Pg = wk.tile([128, 128], F32, name=f"Pg{mi}")
                nc.vector.tensor_scalar(out=Pg[:], in0=scns[mi][:],
                                        scalar1=rowoff3[:, mi : mi + 1],
                                        scalar2=None, op0=Alu.add)
                Pgs.append(Pg)

            chunks = [(0, 0), (1, 0), (1, 1), (1, 2), (2, 0), (2, 1), (2, 2)]
            idxall = wk.tile([128, 7], I32, name="idxall")
            for j, (mi, cch) in enumerate(chunks):
                ch = chains[j % 3]
                sio = siota3[:, cch : cch + 1]
                if cch == 0:
                    roshift = rowoff3[:, mi : mi + 1]
                else:
                    rosh = wk.tile([128, 1], F32, name=f"rosh{j}")
                    nc.vector.tensor_scalar(out=rosh[:], in0=rowoff3[:, mi : mi + 1],
                                            scalar1=float(-128 * cch), scalar2=None,
                                            op0=Alu.add)
                    roshift = rosh[:]
                cmp1 = wk.tile([128, 128], F32, name=f"c1_{j}")
                nc.vector.tensor_scalar(out=cmp1[:], in0=iotaF[:], scalar1=roshift,
                                        scalar2=None, op0=Alu.is_ge)
                rcrow_ps = ch[0:1, 384:512]
                nc.tensor.matmul(rcrow_ps, onesc[:], cmp1[:], start=True, stop=True)
                rcrow = wk.tile([1, 128], F32, name=f"rcr{j}")
                nc.scalar.copy(rcrow[:], rcrow_ps)
                rcb_ps = ch[:, 0:128]
                nc.tensor.matmul(rcb_ps, onesr[:], rcrow[:], start=True, stop=True)
                Omat = wk.tile([128, 128], F32, name=f"om{j}")
                nc.vector.tensor_scalar(out=Omat[:], in0=rcb_ps, scalar1=riota1[:],
                                        scalar2=None, op0=Alu.is_equal)
                prow_ps = ch[:, 128:256]
                nc.tensor.matmul(prow_ps, Omat[:], Pgs[mi][:], start=True, stop=True)
                rcc_ps = ch[:, 256:257]
                nc.tensor.matmul(rcc_ps, Omat[:], riota1[:], start=True, stop=True)
                cmp2 = wk.tile([128, 128], F32, name=f"c2_{j}")
                nc.vector.tensor_scalar(out=cmp2[:], in0=prow_ps, scalar1=sio,
                                        scalar2=None, op0=Alu.is_le)
                wc = wk.tile([128, 1], F32, name=f"wc{j}")
                nc.vector.tensor_reduce(wc[:], cmp2[:], axis=AX, op=Alu.add)
                idx = wk.tile([128, 1], F32, name=f"idx{j}")
                nc.vector.tensor_scalar(out=idx[:], in0=rcc_ps, scalar1=128.0,
                                        scalar2=-128.0, op0=Alu.mult, op1=Alu.add)
                nc.vector.tensor_tensor(out=idx[:], in0=idx[:], in1=wc[:], op=Alu.add)
                vm = wk.tile([128, 1], F32, name=f"vm{j}")
                nc.vector.tensor_scalar(out=vm[:], in0=sio,
                                        scalar1=clb[:, mi : mi + 1],
                                        scalar2=None, op0=Alu.is_ge)
                nc.vector.scalar_tensor_tensor(out=idx[:], in0=vm[:], scalar=50000.0,
                                               in1=idx[:], op0=Alu.mult, op1=Alu.add)
                nc.vector.tensor_copy(idxall[:, j : j + 1], idx[:])

            gat = wk.tile([128, 7, 128], F16, name="gat")
            nc.vector.memset(gat[:], 0.0)
            if MULTI_OFFSET_GATHER:
                nc.gpsimd.indirect_dma_start(
                    out=gat[:],
                    out_offset=None,
                    in_=embp,
                    in_offset=bass.IndirectOffsetOnAxis(ap=idxall[:, 0:7], axis=0),
                    bounds_check=NPIX - 1,
                    oob_is_err=False,
                )
            else:
                for j in range(7):
                    nc.gpsimd.indirect_dma_start(
                        out=gat[:, j, :],
                        out_offset=None,
                        in_=embp,
                        in_offset=bass.IndirectOffsetOnAxis(
                            ap=idxall[:, j : j + 1], axis=0),
                        bounds_check=NPIX - 1,
                        oob_is_err=False,
                    )
            # normalize gathered rows (eps 1e-12); squares alternate DVE/Act
            for j in range(7):
                gv = gat[:, j, :]
                ssq = wk.tile([128, 1], F32, name=f"gsq{j}")
                scr = wk.tile([128, 128], F16, name=f"gsc{j}")
                if j % 2 == 0:
                    nc.scalar.activation(scr[:], gv, Act.Square, accum_out=ssq[:])
                else:
                    nc.vector.scalar_tensor_tensor(out=scr[:], in0=gv, scalar=1.0,
                                                   in1=gv, op0=Alu.mult, op1=Alu.mult,
                                                   accum_out=ssq[:])
                nc.scalar.sqrt(ssq[:], ssq[:])
                nc.vector.tensor_scalar(out=ssq[:], in0=ssq[:], scalar1=1e-12,
                                        scalar2=None, op0=Alu.max)
                nc.vector.reciprocal(ssq[:], ssq[:])
                nc.vector.tensor_scalar(out=gv, in0=gv, scalar1=ssq[:],
                                        scalar2=None, op0=Alu.mult)
            # payload writes (slot-major)
            nc.sync.dma_start(pay_d[1:101, :], gat[0:100, 0, :])
            nc.sync.dma_start(pay_d[101:229, :], gat[:, 1, :])
            nc.sync.dma_start(pay_d[229:357, :], gat[:, 2, :])
            nc.sync.dma_start(pay_d[357:434, :], gat[0:77, 3, :])
            nc.sync.dma_start(pay_d[434:562, :], gat[:, 4, :])
            nc.sync.dma_start(pay_d[562:690, :], gat[:, 5, :])
            nc.sync.dma_start(pay_d[690:767, :], gat[0:77, 6, :])

            # ---------- AllGather ----------
            nc.gpsimd.collective_compute(
                "AllGather", Alu.bypass, replica_groups=groups,
                ins=[pay_d], outs=[gath_d],
            )

            # ---------- static normalize (eps 1e-8) -> f16 dests ----------
            def norm_rows(src_ap, dst_ap, rn, eps, j):
                ssq = wk.tile([128, 1], F32, name=f"ssq{j}")
                scr = wk.tile([128, 128], F16, name=f"nsc{j}")
                nc.scalar.activation(scr[0:rn, :], src_ap, Act.Square,
                                     accum_out=ssq[0:rn, :])
                nc.scalar.sqrt(ssq[0:rn, :], ssq[0:rn, :])
                nc.vector.tensor_scalar(out=ssq[0:rn, :], in0=ssq[0:rn, :],
                                        scalar1=eps, scalar2=None, op0=Alu.max)
                nc.vector.reciprocal(ssq[0:rn, :], ssq[0:rn, :])
                nc.vector.tensor_scalar(out=dst_ap, in0=src_ap,
                                        scalar1=ssq[0:rn, :], scalar2=None,
                                        op0=Alu.mult)

            # pos own slice: normalize f32, cast into mega[:,1,:]
            posn = wk.tile([PC, 128], F32, name="posn")
            norm_rows(posl[0:PC, :], posn[0:PC, :], PC, 1e-8, "p")
            nc.vector.tensor_copy(mega[0:PC, 1, :], posn[0:PC, :])
            # neg chunks 0..2: normalize f32, cast into mega[:,2..4,:]
            for i in range(3):
                nn = wk.tile([128, 128], F32, name=f"nn{i}")
                norm_rows(ng[i][:], nn[:], 128, 1e-8, f"n{i}")
                nc.vector.tensor_copy(mega[:, 2 + i, :], nn[:])
            # neg chunks 3..7 -> f32 tiles, transpose early (f32 PE only)
            UnegT = wk.tile([128, M], F32, name="UnegT")
            for i in range(3, 8):
                r0 = 128 * i
                rn = min(128, M - r0)
                nh = wk.tile([128, 128], F32, name=f"nh{i}")
                norm_rows(ng[i][0:rn, :], nh[0:rn, :], rn, 1e-8, f"n{i}")
                tp = ps.tile([128, 128], F32, name=f"tpn{i}", tag=f"tsp{i % 2}")
                nc.tensor.transpose(tp[0:128, 0:rn], nh[0:rn, :], identf[0:rn, 0:rn])
                nc.scalar.copy(UnegT[:, r0 : r0 + rn], tp[0:128, 0:rn])



            # ---------- compose (batched math) ----------
            cnt16 = wk.tile([8, 3], F16, name="cnt16")
            gath3 = gath_d.rearrange("(a b) d -> a b d", b=PAY)
            nc.sync.dma_start(cnt16[:], gath3[0:8, 0:1, 0:3])
            cnts8 = wk.tile([8, 3], F32, name="cnts8")
            nc.vector.tensor_copy(cnts8[:], cnt16[:])
            # totals -> final counts -> broadcast
            tot_ps = misc[0:1, 9:12]
            nc.tensor.matmul(tot_ps, onesc[0:8, :], cnts8[:], start=True, stop=True)
            cntf = wk.tile([1, 3], F32, name="cntf")
            nc.vector.tensor_tensor(out=cntf[:], in0=tot_ps, in1=kvec[:],
                                    op=Alu.min)
            cfb_ps = misc[:, 12:15]
            nc.tensor.matmul(cfb_ps, onesr[:], cntf[:], start=True, stop=True)
            cntfb = wk.tile([128, 3], F32, name="cntfb")
            nc.scalar.copy(cntfb[:], cfb_ps)
            cntfb5 = wk.tile([128, 5], F32, name="cntfb5")
            nc.vector.tensor_copy(cntfb5[:, 0:3], cntfb[:])
            nc.vector.tensor_copy(cntfb5[:, 3:5],
                                  cntfb[:, 2:3].to_broadcast([128, 2]))
            # per-mask prefix rows + shifted-count rows (direct [1,8] matmuls)
            pr_sb, w_sb = [], []
            for mi in range(3):
                pr_ps = misc[0:1, 18 + 8 * mi : 26 + 8 * mi]
                nc.tensor.matmul(pr_ps, cnts8[:, mi : mi + 1], trils[0:8, 0:8],
                                 start=True, stop=True)
                prm = wk.tile([1, 8], F32, name=f"prm{mi}")
                nc.scalar.copy(prm[:], pr_ps)
                pr_sb.append(prm)
                cs_ps = misc[0:1, 42 + 8 * mi : 50 + 8 * mi]
                nc.tensor.matmul(cs_ps, cnts8[:, mi : mi + 1], shift8[0:8, 0:8],
                                 start=True, stop=True)
                wm = wk.tile([1, 8], F32, name=f"wm{mi}")
                nc.vector.tensor_scalar(out=wm[:], in0=cs_ps, scalar1=-1.0,
                                        scalar2=float(PAY), op0=Alu.mult, op1=Alu.add)
                w_sb.append(wm)
            # broadcast down partitions per chunk: masks [0,1,2,2,2]
            CMASK = [0, 1, 2, 2, 2]
            pBcat = misc[:, 66:106]
            wcat = misc[:, 106:146]
            for q in range(5):
                nc.tensor.matmul(misc[:, 66 + 8 * q : 74 + 8 * q], onesr[:],
                                 pr_sb[CMASK[q]][:], start=True, stop=True)
                nc.tensor.matmul(misc[:, 106 + 8 * q : 114 + 8 * q], onesr[:],
                                 w_sb[CMASK[q]][:], start=True, stop=True)
            cmpP = wk.tile([128, 40], F32, name="cmpP")
            nc.vector.tensor_tensor(out=cmpP[:], in0=sio5x8[:], in1=pBcat,
                                    op=Alu.is_ge)
            tall = wk.tile([128, 40], F32, name="tall")
            nc.vector.tensor_tensor(out=tall[:], in0=cmpP[:], in1=wcat, op=Alu.mult)
            red5 = wk.tile([128, 5], F32, name="red5")
            nc.vector.tensor_reduce(red5[:], tall[:].rearrange("p (q c) -> p q c", q=5),
                                    axis=AX, op=Alu.add)
            src5 = wk.tile([128, 5], F32, name="src5")
            nc.vector.tensor_tensor(out=src5[:], in0=red5[:], in1=sioadj[:],
                                    op=Alu.add)
            vm5 = wk.tile([128, 5], F32, name="vm5")
            nc.vector.tensor_tensor(out=vm5[:], in0=sio5raw[:], in1=cntfb5[:],
                                    op=Alu.is_ge)
            nc.vector.scalar_tensor_tensor(out=src5[:], in0=vm5[:], scalar=100000.0,
                                           in1=src5[:], op0=Alu.mult, op1=Alu.add)
            srcall = wk.tile([128, 5], I32, name="srcall")
            nc.vector.tensor_copy(srcall[:], src5[:])

            if MULTI_OFFSET_GATHER:
                nc.gpsimd.indirect_dma_start(
                    out=mega[:, 0:5, :],
                    out_offset=None,
                    in_=gath_d,
                    in_offset=bass.IndirectOffsetOnAxis(ap=srcall[:, 0:5], axis=0),
                    bounds_check=GA - 1,
                    oob_is_err=False,
                )
            else:
                for q in range(5):
                    nc.gpsimd.indirect_dma_start(
                        out=mega[:, q, :],
                        out_offset=None,
                        in_=gath_d,
                        in_offset=bass.IndirectOffsetOnAxis(
                            ap=srcall[:, q : q + 1], axis=0),
                        bounds_check=GA - 1,
                        oob_is_err=False,
                    )

            # ---------- transposes + sims (all f32 through PE, pipelined) ----------
            megaf = wk.tile([128, 5, 128], F32, name="megaf")
            # anc
            nc.vector.tensor_copy(megaf[:, 0, :], mega[:, 0, :])
            ancT_ps = ps.tile([128, 128], F32, name="ancTps", tag="tsp0")
            nc.tensor.transpose(ancT_ps[:], megaf[:, 0, :], identf[:])
            ancT = wk.tile([128, 128], F32, name="ancT")
            nc.scalar.copy(ancT[:], ancT_ps[:])
            # static negsim columns only need ancT
            nc.tensor.matmul(negsim[:, 384:512], ancT[:, 0:100], UnegT[:, 384:512],
                             start=True, stop=True)
            nc.tensor.matmul(negsim[:, 512:M], ancT[:, 0:100], UnegT[:, 512:M],
                             start=True, stop=True)
            # pos
            nc.vector.tensor_copy(megaf[:, 1, :], mega[:, 1, :])
            upT_ps = ps.tile([128, 128], F32, name="upTps", tag="tsp1")
            nc.tensor.transpose(upT_ps[0:128, 0:PC], megaf[0:PC, 1, :],
                                identf[0:PC, 0:PC])
            UposT = wk.tile([128, PC], F32, name="UposT")
            nc.scalar.copy(UposT[:], upT_ps[0:128, 0:PC])
            nc.tensor.matmul(possim[:], ancT[:, 0:100], UposT[:], start=True, stop=True)
            # neg updated chunks
            for i in range(3):
                nc.vector.tensor_copy(megaf[:, 2 + i, :], mega[:, 2 + i, :])
                tp = ps.tile([128, 128], F32, name=f"tpm{i}", tag=f"tsp{i % 2}")
                nc.tensor.transpose(tp[:], megaf[:, 2 + i, :], identf[:])
                nc.scalar.copy(UnegT[:, 128 * i : 128 * (i + 1)], tp[:])
            nc.tensor.matmul(negsim[:, 0:384], ancT[:, 0:100], UnegT[:, 0:384],
                             start=True, stop=True)

            # ---------- pairwise ----------
            # DVE lane identity: sum_n relu(x - y_n) = sum_n max(x, y_n) - sum_n y_n
            # (tensor_scalar with accum_out: out = op0(in0,s1), accum = op1-reduce)
            ybuf = wk.tile([100, M], F16, name="ybuf")
            nc.vector.tensor_copy(ybuf[:], negsim[:])
            nbuf = wk.tile([100, M], F16, name="nbuf")
            nc.scalar.mul(nbuf[:], negsim[:], -1.0)
            ysum = wk.tile([100, 1], F32, name="ysum")
            nc.vector.tensor_reduce(ysum[:], negsim[:], axis=AX, op=Alu.add)
            validA = wk.tile([128, 1], F32, name="validA")
            nc.vector.tensor_scalar(out=validA[0:100, :], in0=rowiota[0:100, :],
                                    scalar1=cntfb[0:100, 0:1], scalar2=None,
                                    op0=Alu.is_lt)
            amod = wk.tile([100, PC], F32, name="amod")
            nc.vector.tensor_scalar(out=amod[:], in0=possim[:],
                                    scalar1=MARGIN + 4.0, scalar2=None, op0=Alu.add)
            nc.vector.tensor_scalar(out=amod[:], in0=amod[:],
                                    scalar1=validA[0:100, :], scalar2=4.0,
                                    op0=Alu.mult, op1=Alu.subtract)

            accD = wk.tile([100, 128], F32, name="accD")
            nc.vector.memset(accD[:], 0.0)
            accA = wk.tile([100, 128], F32, name="accA")
            nc.vector.memset(accA[:], 0.0)
            accP = wk.tile([100, 128], F32, name="accP")
            nc.gpsimd.memset(accP[:], 0.0)
            scrDs = [wk.tile([100, M], F16, name=f"scrD{k}") for k in range(2)]
            scrAs = [wk.tile([100, M], F16, name=f"scrA{k}") for k in range(2)]
            nD = nA = nP = 0
            for i in range(PC):
                a = amod[:, i : i + 1]
                if i < DVE_N:
                    nc.vector.tensor_scalar(out=scrDs[nD % 2][:], in0=ybuf[:],
                                            scalar1=a, scalar2=None, op0=Alu.max,
                                            op1=Alu.add,
                                            accum_out=accD[:, nD : nD + 1])
                    nD += 1
                else:
                    nc.scalar.activation(scrAs[nA % 2][:], nbuf[:], Act.Relu, bias=a,
                                         scale=1.0, accum_out=accA[:, nA : nA + 1])
                    nA += 1

            # ---------- final reduction ----------
            r3 = wk.tile([100, 3], F32, name="r3")
            nc.vector.tensor_reduce(r3[:, 0:1], accD[:, 0 : max(nD, 1)], axis=AX,
                                    op=Alu.add)
            # subtract nD * sum_n y_n (max-identity correction for DVE lane)
            nc.vector.scalar_tensor_tensor(out=r3[:, 0:1], in0=ysum[:],
                                           scalar=float(-nD), in1=r3[:, 0:1],
                                           op0=Alu.mult, op1=Alu.add)
            nc.vector.tensor_reduce(r3[:, 1:2], accA[:, 0 : max(nA, 1)], axis=AX,
                                    op=Alu.add)
            nc.vector.tensor_reduce(r3[:, 2:3], accP[:, 0 : max(nP, 1)], axis=AX,
                                    op=Alu.add)
            rsum = wk.tile([100, 1], F32, name="rsum")
            nc.vector.tensor_reduce(rsum[:], r3[:], axis=AX, op=Alu.add)
            tot2 = misc[0:1, 146:147]
            nc.tensor.matmul(tot2, rsum[:], onesc[0:100, :], start=True, stop=True)
            den = wk.tile([1, 1], F32, name="den")
            nc.vector.tensor_scalar(out=den[:], in0=cntf[:, 0:1], scalar1=1.0,
                                    scalar2=1e6, op0=Alu.max, op1=Alu.mult)
            nc.vector.reciprocal(den[:], den[:])
            res = wk.tile([1, 1], F32, name="res")
            nc.vector.tensor_tensor(out=res[:], in0=tot2, in1=den[:], op=Alu.mult)
            nc.sync.dma_start(out_d, res[:])

    return nc


def _host_shards(preds, embeddings, fsss_gts, pos_memory, neg_memory):
    identf = np.eye(128, dtype=np.float32)
    trils = np.tril(np.ones((128, 128), np.float32), -1).T  # lhsT[k,m]=1 iff k<m
    riota1 = (np.arange(128, dtype=np.float32) + 1.0).reshape(128, 1)
    rowiota = np.arange(128, dtype=np.float32).reshape(128, 1)
    iotaf = np.tile(np.arange(128, dtype=np.float32), (128, 1))
    siota3 = np.stack([np.arange(128, dtype=np.float32) + 128 * c for c in range(3)],
                      axis=1)
    kvec = np.array([[KA, KP, KP]], np.float32)
    shift8 = np.zeros((8, 8), np.float32)
    for k in range(7):
        shift8[k, k + 1] = 1.0

    in_maps = []
    for c in range(NCORES):
        psub = preds[c, :, ::4, ::4]  # [21,128,128]
        preds_t = np.ascontiguousarray(
            psub.transpose(1, 2, 0).reshape(128, C * 128)
        ).astype(np.float16)
        gts_t = np.ascontiguousarray(fsss_gts[c, ::4, ::4]).astype(np.int32)
        embp = np.ascontiguousarray(
            embeddings[c].transpose(1, 2, 0).reshape(NPIX, D)
        ).astype(np.float16)
        posown = np.ascontiguousarray(
            pos_memory[PC * c : PC * (c + 1)], dtype=np.float32
        )
        p_ = np.arange(128, dtype=np.float32)
        siopos = p_ + PC * c
        siopos[PC:] = 1e6
        sio5raw = np.stack([p_, siopos, p_, p_ + 128, p_ + 256], axis=1)
        secq = np.array([SEC[0], SEC[1], SEC[2], SEC[2], SEC[2]], np.float32)
        sioadj = sio5raw + (secq - PAY)[None, :]
        sio5x8 = np.repeat(sio5raw, 8, axis=1)
        in_maps.append(
            {
                "preds_t": preds_t,
                "gts_t": gts_t,
                "embp": embp,
                "posown": posown,
                "negmem": np.ascontiguousarray(neg_memory, dtype=np.float32),
                "identf": identf,
                "trils": trils.astype(np.float32),
                "riota1": riota1,
                "rowiota": rowiota,
                "iotaf": np.ascontiguousarray(iotaf),
                "siota3": np.ascontiguousarray(siota3),
                "shift8": shift8,
                "sio5x8": np.ascontiguousarray(sio5x8),
                "sio5raw": np.ascontiguousarray(sio5raw),
                "sioadj": np.ascontiguousarray(sioadj),
                "kvec": kvec,
            }
        )
    return in_maps


def kernel(preds, embeddings, fsss_gts, pos_memory, neg_memory):
    global LAST_EXEC_NS
    _install_patches()
    from concourse.bass_utils import run_bass_kernel_spmd

    if "nc" not in _cache:
        _cache["nc"] = _build_module()
    nc = _cache["nc"]

    in_maps = _host_shards(
        np.asarray(preds), np.asarray(embeddings), np.asarray(fsss_gts),
        np.asarray(pos_memory), np.asarray(neg_memory),
    )
    res = run_bass_kernel_spmd(nc, in_maps, list(range(NCORES)), trace=TRACE)
    LAST_EXEC_NS = res.exec_time_ns
    total = np.float32(0.0)
    for r in res.results:
        total = total + r["out"][0, 0]
    return np.float32(total)
